# revision 1
# baseline (speedup 1.0000x reference)
"""Trainium2 Bass kernel for Swin-style window attention (MegatronWindowAttention).

Full computation per window w (49 tokens, dim 256, 8 heads x 32):
  qkv = x @ qkv_w.T + qkv_b ; q,k,v per head
  attn = softmax(q*scale @ k.T + bias + mask[w%64]) ; out = (attn @ v) @ proj_w.T + proj_b

Sharding: data-parallel over the window-batch dim B_=4096 across 8 cores (512 windows each).

Device dataflow per core (4 blocks of 128 windows = 6272 tokens):
  - PE-transpose x tiles -> x.T bf16 (channels on partitions)
  - QKV: W.T-stationary matmuls, tokens moving -> Q.T/K.T/V.T bf16 [oc, t] resident
  - per window-pair (98 tokens): PE-transpose V slices -> V [token, d] padded layout;
    S.T = K.T^T Q.T per (head, window) packed with tile_position; exp on ACT (PSUM->SBUF);
    * exp(bias+mask) multiplicative fold; AV matmuls give O [i, d] + rowsum column
    (ones-augmented V); normalize via DVE reciprocal+mul; PE-transpose O -> O.T;
    proj matmuls; + proj_b; DMA out.
"""

import numpy as np
import ml_dtypes
from contextlib import ExitStack

import concourse.bass as bass
import concourse.tile as tile
import concourse.mybir as mybir
from concourse import bacc
from concourse import bass_utils

WH = WW = 7
NTOK = 49
DIM = 256
NH = 8
HD = 32
SCALE = HD ** -0.5
NCORES = 8
B_FULL = 4096
NW = 64
B_CORE = B_FULL // NCORES          # 512 windows per core
T_CORE = B_CORE * NTOK             # 25088 tokens
NBLOCK = 4
W_BLK = 128                        # windows per block
T_BLK = W_BLK * NTOK               # 6272 tokens per block
T_PAD = T_BLK + 16                 # padded cols so 64-wide window reads stay in-bounds
NPAIR = W_BLK // 2                 # 64 pairs per block
QKV_CHUNK = 448                    # token chunk for qkv matmuls (6272 = 14*448)
NCHUNK = T_BLK // QKV_CHUNK

F32 = mybir.dt.float32
BF16 = mybir.dt.bfloat16
AF = mybir.ActivationFunctionType


def _rel_pos_index():
    coords = np.stack(np.meshgrid(np.arange(WH), np.arange(WW), indexing='ij'))
    flat = coords.reshape(2, -1)
    rel = flat[:, :, None] - flat[:, None, :]
    rel = rel.transpose(1, 2, 0).copy()
    rel[:, :, 0] += WH - 1
    rel[:, :, 1] += WW - 1
    rel[:, :, 0] *= 2 * WW - 1
    return rel.sum(-1).reshape(-1)


def build_kernel(ctx: ExitStack, tc: tile.TileContext, ins: dict, out_ap: bass.AP):
    nc = tc.nc
    x = ins["x"]          # [T_CORE, 256] f32
    qkvwt = ins["qkvwt"]  # [2, 6, 128, 128] bf16
    qkvb = ins["qkvb"]    # [128, 6] f32
    pwt = ins["pwt"]      # [2, 128, 256] bf16
    pbb = ins["pbb"]      # [128, 256] f32
    expb = ins["expb"]    # [32, 113, 392] bf16
    idf = ins["identf"]   # [128, 128] f32
    idb = ins["identb"]   # [128, 128] bf16

    const = ctx.enter_context(tc.tile_pool(name="const", bufs=1))
    qkvwt_sb = const.tile([128, 2 * 6 * 128], BF16, tag="qkvwt")
    nc.sync.dma_start(qkvwt_sb[:], qkvwt.rearrange("p a b c -> p (a b c)"))
    qkvwt_v = qkvwt_sb[:].rearrange("p (a b c) -> p a b c", a=2, b=6)
    qkvb_sb = const.tile([128, 6], F32, tag="qkvb")
    nc.sync.dma_start(qkvb_sb[:], qkvb[:])
    pwt_sb = const.tile([128, 2 * 256], BF16, tag="pwt")
    nc.sync.dma_start(pwt_sb[:], pwt.rearrange("p a c -> p (a c)"))
    pbb_sb = const.tile([128, 256], F32, tag="pbb")
    nc.sync.dma_start(pbb_sb[:], pbb[:])
    expb_sb = const.tile([128, 32 * 392], BF16, tag="expb")
    nc.sync.dma_start(expb_sb[:], expb.rearrange("p m f -> p (m f)"))
    idf_sb = const.tile([128, 128], F32, tag="identf")
    nc.sync.dma_start(idf_sb[:], idf[:])
    idb_sb = const.tile([128, 128], BF16, tag="identb")
    nc.sync.dma_start(idb_sb[:], idb[:])

    xt_pool = ctx.enter_context(tc.tile_pool(name="xt", bufs=1))
    qkv_pool = ctx.enter_context(tc.tile_pool(name="qkvt", bufs=1))
    xin_pool = ctx.enter_context(tc.tile_pool(name="xin", bufs=3))
    v_pool = ctx.enter_context(tc.tile_pool(name="vsb", bufs=2))
    p_pool = ctx.enter_context(tc.tile_pool(name="psb", bufs=2))
    o_pool = ctx.enter_context(tc.tile_pool(name="osb", bufs=2))
    ot_pool = ctx.enter_context(tc.tile_pool(name="otsb", bufs=2))
    r_pool = ctx.enter_context(tc.tile_pool(name="rsb", bufs=2))
    out_pool = ctx.enter_context(tc.tile_pool(name="outsb", bufs=3))

    psA = ctx.enter_context(tc.tile_pool(name="psA", bufs=2, space="PSUM"))
    psS = ctx.enter_context(tc.tile_pool(name="psS", bufs=1, space="PSUM"))
    psW = ctx.enter_context(tc.tile_pool(name="psW", bufs=1, space="PSUM"))

    for blk in range(NBLOCK):
        t0 = blk * T_BLK
        # ---- phase L: load + transpose x -> Xt (bf16, [ic, t]) ----
        Xt = [qkv_pool.tile([128, T_BLK], BF16, tag=f"xt{kb}", name=f"xt{kb}_{blk}") for kb in range(2)]
        for i in range(T_BLK // 128):
            xsb = xin_pool.tile([128, 256], F32, tag="xin")
            nc.sync.dma_start(xsb[:], x[t0 + 128 * i: t0 + 128 * (i + 1), :])
            for kb in range(2):
                tp = psA.tile([128, 448], F32, tag="psA")
                nc.tensor.transpose(tp[:, 0:128], xsb[:, 128 * kb:128 * (kb + 1)], idf_sb[:])
                nc.scalar.copy(Xt[kb][:, 128 * i:128 * (i + 1)], tp[:, 0:128])
        # ---- phase Q: qkv matmuls -> QKVt (bf16, [oc, t]) ----
        QKVt = [qkv_pool.tile([128, T_PAD], BF16, tag=f"qkv{ob}", name=f"qkv{ob}_{blk}") for ob in range(6)]
        for ob in range(6):
            nc.vector.memset(QKVt[ob][:, T_BLK:T_PAD], 0.0)
        for c in range(NCHUNK):
            cs = slice(QKV_CHUNK * c, QKV_CHUNK * (c + 1))
            for ob in range(6):
                qps = psA.tile([128, 448], F32, tag="psA")
                for kb in range(2):
                    nc.tensor.matmul(qps[:], qkvwt_v[:, kb, ob, :], Xt[kb][:, cs],
                                     start=(kb == 0), stop=(kb == 1))
                sc = SCALE if ob in (2, 3) else 1.0
                nc.scalar.activation(QKVt[ob][:, cs], qps[:], AF.Identity,
                                     bias=qkvb_sb[:, ob:ob + 1], scale=sc)
        # ---- phase A: attention per window pair ----
        for u in range(NPAIR):
            c0 = 98 * u
            # V relayout: PE transpose V.T slices -> Vps [t(pad64), oc], evac strided
            vps = psW.tile([128, 256], BF16, tag="psW", name=f"vps_{blk}_{u}")
            for w in range(2):
                for kb in range(2):
                    nc.tensor.transpose(
                        vps[64 * w:64 * w + 64, 128 * kb:128 * (kb + 1)],
                        QKVt[4 + kb][:, c0 + 49 * w: c0 + 49 * w + 64],
                        idb_sb[:], tile_position=(0, 64 * w))
            vsb = v_pool.tile([128, 264], BF16, tag="vsb")
            vsb_v = vsb[:].rearrange("p (h c) -> p h c", h=8)
            vps_v = vps[:].rearrange("p (h c) -> p h c", h=8)
            nc.scalar.copy(vsb_v[:, :, 0:32], vps_v[:, :, :])
            nc.vector.memset(vsb_v[:, :, 32:33], 1.0)
            # S.T matmuls: per (head, window)
            sps = psS.tile([128, 2048], F32, tag="psS")
            for h in range(NH):
                kt = QKVt[2 + h // 4]
                qt = QKVt[0 + h // 4]
                rs = slice(32 * (h % 4), 32 * (h % 4) + 32)
                sc0 = 512 * (h % 4) + 49 * (h // 4)
                for w in range(2):
                    ws64 = slice(c0 + 49 * w, c0 + 49 * w + 64)
                    ws = slice(c0 + 49 * w, c0 + 49 * w + 49)
                    nc.tensor.matmul(
                        sps[64 * w:64 * w + 64, sc0:sc0 + 49],
                        kt[rs, ws64], qt[rs, ws], start=True, stop=True,
                        tile_position=(32 * (h % 4), 64 * w))
            # exp (ACT, PSUM->SBUF) then * exp(bias+mask) (DVE)
            pexp = p_pool.tile([128, 392], BF16, tag="pexp")
            sps_v = sps[:].rearrange("p (b c) -> p b c", b=4)[:, :, 0:98]
            pexp_v = pexp[:].rearrange("p (b c) -> p b c", b=4)
            nc.scalar.activation(pexp_v[:, :, :], sps_v, AF.Exp)
            pm = p_pool.tile([128, 408], BF16, tag="pm")
            nc.vector.tensor_mul(pm[:, 0:392], pexp[:],
                                 expb_sb[:, 392 * (u % 32):392 * (u % 32 + 1)])
            nc.vector.memset(pm[:, 392:408], 1.0)
            # AV: O[i, d] + rowsum col; lhsT = P.T slice, rhs = V(+ones)
            avps = psW.tile([128, 1024], F32, tag="psW", name=f"avps_{blk}_{u}")
            for h in range(NH):
                pc = 98 * (h % 4) + 49 * (h // 4)
                for w in range(2):
                    nc.tensor.matmul(
                        avps[64 * w:64 * w + 64, 512 * w + 33 * h:512 * w + 33 * h + 33],
                        pm[64 * w:64 * w + 49, pc:pc + 64],
                        vsb[64 * w:64 * w + 49, 33 * h:33 * h + 33],
                        start=True, stop=True, tile_position=(64 * w, 64 * w))
            # normalize: recip of rowsum col, broadcast-mult -> Onorm dense [113, 256]
            recip = r_pool.tile([128, 8], F32, tag="recip")
            onorm = o_pool.tile([128, 256], BF16, tag="onorm")
            onorm_v = onorm[:].rearrange("p (h c) -> p h c", h=8)
            for w in range(2):
                pr = slice(64 * w, 64 * w + 64)
                av_w = avps[pr, 512 * w:512 * w + 264].rearrange("p (h c) -> p h c", h=8)
                nc.vector.reciprocal(recip[pr, :], av_w[:, :, 32])
                recip_b = recip[pr, :].unsqueeze(2).broadcast_to([64, 8, 32])
                nc.vector.tensor_mul(onorm_v[pr, :, :], av_w[:, :, 0:32], recip_b)
            # transpose O -> O.T chunks [128 hd, 49] ; cols 49*(2*hb+w)
            otps = psW.tile([128, 2048], BF16, tag="psW", name=f"otps_{blk}_{u}")
            for w in range(2):
                for hb in range(2):
                    nc.tensor.transpose(
                        otps[:, 1024 * w + 512 * hb:1024 * w + 512 * hb + 49],
                        onorm[64 * w:64 * w + 49, 128 * hb:128 * (hb + 1)],
                        idb_sb[64 * w:64 * w + 49, 64 * w:64 * w + 49],
                        tile_position=(64 * w, 0))
            otsb = ot_pool.tile([128, 196], BF16, tag="otsb")
            otps_v = otps[:].rearrange("p (w b c) -> p w b c", w=2, b=2)[:, :, :, 0:49]
            otsb_v = otsb[:].rearrange("p (b w c) -> p w b c", b=2, w=2)
            nc.scalar.copy(otsb_v[:, :, :, :], otps_v)
            # proj: lhsT = O.T block stationary, rhs = proj_w.T moving
            pps = psW.tile([98, 256], F32, tag="psW", name=f"pps_{blk}_{u}")
            for hb in range(2):
                nc.tensor.matmul(pps[:], otsb[:, 98 * hb:98 * (hb + 1)],
                                 pwt_sb[:, 256 * hb:256 * (hb + 1)],
                                 start=(hb == 0), stop=(hb == 1))
            osb = out_pool.tile([98, 256], F32, tag="outsb")
            nc.vector.tensor_add(osb[:], pps[:], pbb_sb[0:98, :])
            nc.sync.dma_start(out_ap[t0 + c0:t0 + c0 + 98, :], osb[:])


_CACHED = {}


def _get_program():
    if "nc" in _CACHED:
        return _CACHED["nc"]
    nc = bacc.Bacc("TRN2", target_bir_lowering=False, debug=False)
    ins = {
        "x": nc.dram_tensor("x", [T_CORE, DIM], F32, kind="ExternalInput").ap(),
        "qkvwt": nc.dram_tensor("qkvwt", [128, 2, 6, 128], BF16, kind="ExternalInput").ap(),
        "qkvb": nc.dram_tensor("qkvb", [128, 6], F32, kind="ExternalInput").ap(),
        "pwt": nc.dram_tensor("pwt", [128, 2, 256], BF16, kind="ExternalInput").ap(),
        "pbb": nc.dram_tensor("pbb", [128, 256], F32, kind="ExternalInput").ap(),
        "expb": nc.dram_tensor("expb", [128, 32, 392], BF16, kind="ExternalInput").ap(),
        "identf": nc.dram_tensor("identf", [128, 128], F32, kind="ExternalInput").ap(),
        "identb": nc.dram_tensor("identb", [128, 128], BF16, kind="ExternalInput").ap(),
    }
    out_ap = nc.dram_tensor("out", [T_CORE, DIM], F32, kind="ExternalOutput").ap()
    with tile.TileContext(nc) as tc:
        with ExitStack() as ctx:
            build_kernel(ctx, tc, ins, out_ap)
    nc.compile()
    _CACHED["nc"] = nc
    return nc


def _host_prep(mask, qkv_w, qkv_b, proj_w, proj_b, bias_table):
    bf = ml_dtypes.bfloat16
    qkvwt = np.ascontiguousarray(
        qkv_w.reshape(6, 128, 2, 128).transpose(3, 2, 0, 1)).astype(bf)
    qb = np.asarray(qkv_b, np.float32).copy()
    qb[256:512] *= SCALE
    qkvb = np.ascontiguousarray(qb.reshape(6, 128).T)
    pwt = np.ascontiguousarray(np.asarray(proj_w, np.float32).T.reshape(2, 128, 256).transpose(1, 0, 2)).astype(bf)
    pbb = np.ascontiguousarray(np.broadcast_to(np.asarray(proj_b, np.float32), (128, 256)))
    # combined exp(bias + mask), transposed to [j, i], packed per pair pattern
    rel = _rel_pos_index()
    bias_g = np.asarray(bias_table, np.float32)[rel].reshape(NTOK, NTOK, NH)  # [i, j, h]
    comb = bias_g[None].transpose(0, 3, 1, 2) + np.asarray(mask, np.float32)[:, None]  # [64, h, i, j]
    combT = np.exp(comb.transpose(0, 1, 3, 2))  # [64, h, j, i]
    expb = np.zeros((32, 128, 392), np.float32)
    for p in range(32):
        for h in range(NH):
            hc = 98 * (h % 4) + 49 * (h // 4)
            expb[p, 0:49, hc:hc + 49] = combT[2 * p, h]
            expb[p, 64:113, hc:hc + 49] = combT[2 * p + 1, h]
    expb = np.ascontiguousarray(expb.transpose(1, 0, 2)).astype(bf)
    identf = np.eye(128, dtype=np.float32)
    identb = np.eye(128).astype(bf)
    return qkvwt, qkvb, pwt, pbb, expb, identf, identb


def kernel(x, mask, qkv_w, qkv_b, proj_w, proj_b, bias_table, _trace=False):
    x = np.asarray(x, np.float32)
    qkvwt, qkvb, pwt, pbb, expb, identf, identb = _host_prep(
        np.asarray(mask), np.asarray(qkv_w), np.asarray(qkv_b),
        np.asarray(proj_w), np.asarray(proj_b), np.asarray(bias_table))
    xs = x.reshape(B_FULL, NTOK, DIM)
    in_maps = []
    for c in range(NCORES):
        shard = np.ascontiguousarray(
            xs[c * B_CORE:(c + 1) * B_CORE].reshape(T_CORE, DIM))
        in_maps.append({
            "x": shard, "qkvwt": qkvwt, "qkvb": qkvb, "pwt": pwt, "pbb": pbb,
            "expb": expb, "identf": identf, "identb": identb,
        })
    nc = _get_program()
    res = bass_utils.run_bass_kernel_spmd(nc, in_maps, core_ids=list(range(NCORES)),
                                          trace=_trace)
    out = np.stack([r["out"] for r in res.results])  # [8, T_CORE, 256]
    out = out.reshape(B_FULL, NTOK, DIM)
    if _trace:
        kernel.last_results = res
    return out



# revision 25
# speedup vs baseline: 1.3923x; 1.3923x over previous
"""Trainium2 Bass kernel for Swin-style window attention (MegatronWindowAttention).

Per window w (49 tokens, dim 256, 8 heads x 32):
  qkv = x @ qkv_w.T + qkv_b ; attn = softmax(q*scale @ k.T + bias + mask[w%64])
  out = (attn @ v) @ proj_w.T + proj_b

Sharding: data-parallel over B_=4096 windows across 8 cores (512 windows each).

Device dataflow per core (4 blocks of 128 windows; window PAIRS of 98 tokens):
  - x.T bf16 arrives via DMA xbar transpose (host passes bf16 copy of x)
  - Q.T/K.T staged per 8-pair group: W-stationary matmuls (FWL 128-col weights)
  - V per pair in [token, channel] layout: x.T-chunk-stationary matmuls
    streaming Wv.T (slotted 33-wide per head, ones col for rowsum)
  - S.T per (head, pair): one matmul, lhsT = K.T 128-col span (both windows
    compact at partitions 0:98), rhs = Q.T 98 cols; exp on ACT; * exp(bias)
    table (zeros kill cross-window garbage quadrants) on DVE
  - AV per head: lhsT = pm 128-col slot (FWL), rhs = V_aug -> O + rowsum
  - normalize via DVE reciprocal+mul; 2 PE transposes -> O.T; proj 2 matmuls
"""

import os
import numpy as np
import ml_dtypes
from contextlib import ExitStack

KSTAGE = int(os.environ.get("KSTAGE", "5"))

import concourse.bass as bass
import concourse.tile as tile
import concourse.mybir as mybir
from concourse import bacc
from concourse import bass_utils

WH = WW = 7
NTOK = 49
DIM = 256
NH = 8
HD = 32
SCALE = HD ** -0.5
NCORES = 8
B_FULL = 4096
NW = 64
B_CORE = B_FULL // NCORES          # 512 windows per core
T_CORE = B_CORE * NTOK             # 25088 tokens
NBLOCK = 4
W_BLK = 128                        # windows per block
T_BLK = W_BLK * NTOK               # 6272 tokens per block
T_PAD = T_BLK + 32                 # tail pad so 128-wide lhsT reads stay in-bounds
NPAIR = W_BLK // 2                 # 64 pairs per block
GPAIR = 8                          # pairs per QK staging group
NGRP = NPAIR // GPAIR              # 8 groups per block
GW = GPAIR * 98                    # 784 tokens per group
GWH = GW + 32                      # +halo for K 128-col lhsT reads
GCH = GWH // 2                     # 408: QK psum chunk width

F32 = mybir.dt.float32
BF16 = mybir.dt.bfloat16
AF = mybir.ActivationFunctionType
ALU = mybir.AluOpType


def _rel_pos_index():
    coords = np.stack(np.meshgrid(np.arange(WH), np.arange(WW), indexing='ij'))
    flat = coords.reshape(2, -1)
    rel = flat[:, :, None] - flat[:, None, :]
    rel = rel.transpose(1, 2, 0).copy()
    rel[:, :, 0] += WH - 1
    rel[:, :, 1] += WW - 1
    rel[:, :, 0] *= 2 * WW - 1
    return rel.sum(-1).reshape(-1)


def build_kernel(ctx: ExitStack, tc: tile.TileContext, ins: dict, out_ap: bass.AP,
                 mask_zero: bool, pb_zero: bool):
    nc = tc.nc
    xbf = ins["xbf"]        # [T_CORE, 256] bf16
    qkw = ins["qkw"]        # [128, 2, 4, 128] bf16 (ic, kb, ob(q0 q1 k0 k1), oc)
    wv = ins["wv"]          # [128, 2, 264] bf16 (ic, kb, slotted oc)
    pwt = ins["pwt"]        # [128, 2, 256] bf16
    ebias = ins["ebias"]    # [128, 8, 98] bf16 (fast) or placeholder
    idb = ins["identb"]     # [128, 128] bf16
    qkvb = ins["qkvb"]      # [128, 4] f32 per-partition bias for q0 q1 k0 k1
    vbb = ins["vbb"]        # [128, 264] f32 broadcast v-bias (slotted)
    expbm = ins.get("expbm")  # [128, 32*784] bf16 (general path only)
    pbb = ins.get("pbb")    # [128, 256] f32 (general path only)

    const = ctx.enter_context(tc.tile_pool(name="const", bufs=1))
    qkw_sb = const.tile([128, 2 * 4 * 128], BF16, tag="qkw")
    nc.sync.dma_start(qkw_sb[:], qkw.rearrange("p a b c -> p (a b c)"))
    qkw_v = qkw_sb[:].rearrange("p (a b c) -> p a b c", a=2, b=4)
    wv_sb = const.tile([128, 2 * 264], BF16, tag="wv")
    nc.sync.dma_start(wv_sb[:], wv.rearrange("p a c -> p (a c)"))
    wv_v = wv_sb[:].rearrange("p (a c) -> p a c", a=2)
    pwt_sb = const.tile([128, 2 * 256], BF16, tag="pwt")
    nc.sync.dma_start(pwt_sb[:], pwt.rearrange("p a c -> p (a c)"))
    idb_sb = const.tile([128, 128], BF16, tag="identb")
    nc.sync.dma_start(idb_sb[:], idb[:])
    qkvb_sb = const.tile([128, 4], F32, tag="qkvb")
    nc.sync.dma_start(qkvb_sb[:], qkvb[:])
    vbb_sb = const.tile([128, 264], F32, tag="vbb")
    nc.sync.dma_start(vbb_sb[:], vbb[:])
    if mask_zero:
        eb_sb = const.tile([128, 8 * 98], BF16, tag="ebias")
        nc.sync.dma_start(eb_sb[:], ebias.rearrange("p a c -> p (a c)"))
    else:
        eb_sb = const.tile([128, 32 * 784], BF16, tag="expbm")
        nc.sync.dma_start(eb_sb[:], expbm[:])
    if not pb_zero:
        pbb_sb = const.tile([128, 256], F32, tag="pbb")
        nc.sync.dma_start(pbb_sb[:], pbb[:])

    xt_pool = ctx.enter_context(tc.tile_pool(name="xt", bufs=2))
    qk_pool = ctx.enter_context(tc.tile_pool(name="qk", bufs=2))
    pm_pool = ctx.enter_context(tc.tile_pool(name="pm", bufs=2))
    v_pool = ctx.enter_context(tc.tile_pool(name="vsb", bufs=2))
    r_pool = ctx.enter_context(tc.tile_pool(name="rsb", bufs=2))
    on_pool = ctx.enter_context(tc.tile_pool(name="onorm", bufs=2))
    ot_pool = ctx.enter_context(tc.tile_pool(name="otsb", bufs=2))
    out_pool = ctx.enter_context(tc.tile_pool(name="outsb", bufs=3))

    ps_qk = ctx.enter_context(tc.tile_pool(name="psqk", bufs=2, space="PSUM"))
    ps_s = ctx.enter_context(tc.tile_pool(name="pss", bufs=1, space="PSUM"))
    ps_w = ctx.enter_context(tc.tile_pool(name="psw", bufs=2, space="PSUM"))

    for blk in range(NBLOCK):
        t0 = blk * T_BLK
        # ---- x.T loaded directly (host pre-transposed); zero the tail pad ----
        Xt = [xt_pool.tile([128, T_PAD], BF16, tag=f"xt{kb}", name=f"xt{kb}_{blk}")
              for kb in range(2)]
        for kb in range(2):
            nc.vector.memset(Xt[kb][:, T_BLK:T_PAD], 0.0)
            nc.sync.dma_start(Xt[kb][:, 0:T_BLK], xbf[kb, :, t0:t0 + T_BLK])
        for grp in range(NGRP):
            g0 = GW * grp
            # ---- Q.T/K.T staging for this group (halo of 32 for K reads) ----
            # last group's halo reads the zeroed pad region
            qks = [qk_pool.tile([128, GWH], BF16, tag=f"qk{ob}", name=f"qk{ob}_{blk}_{grp}")
                   for ob in range(4)]
            for c2 in range(2):
                cs = slice(g0 + GCH * c2, g0 + GCH * (c2 + 1))
                for ob in range(4):
                    qkp = ps_qk.tile([128, 512], F32, tag="qkp")
                    for kb in range(2):
                        nc.tensor.matmul(qkp[:, 0:GCH], qkw_v[:, kb, ob, :],
                                         Xt[kb][:, cs],
                                         start=(kb == 0), stop=(kb == 1))
                    nc.vector.tensor_scalar_add(
                        qks[ob][:, GCH * c2:GCH * (c2 + 1)], qkp[:, 0:GCH],
                        qkvb_sb[:, ob:ob + 1])
            # ---- attention pairs ----
            for p8 in range(GPAIR):
                u = GPAIR * grp + p8
                c0 = 98 * u            # block-local token offset
                cg = 98 * p8           # group-local token offset
                if KSTAGE <= 1:
                    osb = out_pool.tile([128, 256], F32, tag="outsb")
                    nc.scalar.copy(osb[0:98, :], qks[0][0:98, 0:256])
                    nc.sync.dma_start(out_ap[t0 + c0:t0 + c0 + 98, :], osb[0:98, :])
                    continue
                # S.T: one matmul per head, both windows (keys compact 0:98).
                # Row-band h%4 -> its own PSUM bank (concurrent row tiles must
                # not share a bank); heads h and h+4 share a band (serialized)
                # and pack side by side within the bank.
                sps = ps_s.tile([128, 2048], F32, tag="sps")
                for h in range(NH):
                    kt = qks[2 + h // 4]
                    qt = qks[0 + h // 4]
                    rs = slice(32 * (h % 4), 32 * (h % 4) + 32)
                    sc = 512 * (h % 4) + 98 * (h // 4)
                    nc.tensor.matmul(
                        sps[:, sc:sc + 98],
                        kt[rs, cg:cg + 128], qt[rs, cg:cg + 98],
                        start=True, stop=True, tile_position=(32 * (h % 4), 0))
                # exp (ACT) then * exp(bias[+mask]) (DVE, zeros kill garbage)
                # pm slots packed 98 apart: AV's 128-wide lhsT reads spill
                # into the next slot's valid data (garbage -> unused out
                # partitions 98:128); only the last slot needs a tail memset.
                # pm slot s = 2*(h%4) + h//4 (bank-major); ebias tables are
                # host-reordered to match
                pex = pm_pool.tile([128, 784], BF16, tag="pex")
                pex_v = pex[:].rearrange("p (r c) -> p r c", r=4)
                sps_v = sps[:].rearrange("p (r c) -> p r c", r=4)[:, :, 0:196]
                nc.scalar.activation(pex_v, sps_v, AF.Exp)
                pm = pm_pool.tile([128, 832], BF16, tag="pm")
                pm_v = pm[:, 0:784].rearrange("p (r c) -> p r c", r=4)
                nc.vector.memset(pm[:, 784:814], 0.0)
                if mask_zero:
                    ebv = eb_sb[:].rearrange("p (r c) -> p r c", r=4)
                else:
                    ebv = eb_sb[:, 784 * (u % 32):784 * (u % 32 + 1)].rearrange(
                        "p (r c) -> p r c", r=4)
                nc.vector.tensor_mul(pm_v, pex_v, ebv)
                # V in [token, channel-slot] layout: x.T chunk stationary
                # (emitted after S so the exp overlaps the V matmuls on PE)
                vw = ps_w.tile([128, 512], F32, tag="work", name=f"v_{blk}_{u}")
                for kb in range(2):
                    nc.tensor.matmul(vw[:, 0:264], Xt[kb][:, c0:c0 + 128],
                                     wv_v[:, kb, :], start=(kb == 0), stop=(kb == 1))
                vsb = v_pool.tile([128, 264], BF16, tag="vsb")
                nc.vector.tensor_add(vsb[0:98, :], vw[0:98, 0:264], vbb_sb[0:98, :])
                vsb_v = vsb[:].rearrange("p (h c) -> p h c", h=8)
                nc.vector.memset(vsb_v[0:98, :, 32:33], 1.0)
                if KSTAGE <= 3:
                    osb = out_pool.tile([128, 256], F32, tag="outsb")
                    nc.scalar.copy(osb[0:98, :], pm[0:98, 0:256])
                    nc.sync.dma_start(out_ap[t0 + c0:t0 + c0 + 98, :], osb[0:98, :])
                    continue
                # AV: O[q, d-slot] + rowsum col; lhsT = pm 128-col span (FWL)
                avw = ps_w.tile([128, 512], F32, tag="work", name=f"av_{blk}_{u}")
                for h in range(NH):
                    s_h = 2 * (h % 4) + h // 4
                    nc.tensor.matmul(
                        avw[:, 33 * h:33 * h + 33],
                        pm[0:98, 98 * s_h:98 * s_h + 128],
                        vsb[0:98, 33 * h:33 * h + 33], start=True, stop=True)
                # normalize
                av_v = avw[:, 0:264].rearrange("p (h c) -> p h c", h=8)
                recip = r_pool.tile([128, 8], F32, tag="recip")
                nc.vector.reciprocal(recip[0:98, :], av_v[0:98, :, 32])
                onorm = on_pool.tile([128, 256], BF16, tag="onorm")
                onorm_v = onorm[:].rearrange("p (h c) -> p h c", h=8)
                recip_b = recip[0:98, :].unsqueeze(2).broadcast_to([98, 8, 32])
                nc.vector.tensor_mul(onorm_v[0:98, :, :], av_v[0:98, :, 0:32], recip_b)
                if KSTAGE <= 4:
                    osb = out_pool.tile([128, 256], F32, tag="outsb")
                    nc.scalar.copy(osb[0:98, :], onorm[0:98, :])
                    nc.sync.dma_start(out_ap[t0 + c0:t0 + c0 + 98, :], osb[0:98, :])
                    continue
                # O.T via 2 PE transposes -> psum -> sbuf 128-col slots
                otw = ps_w.tile([128, 512], F32, tag="work", name=f"ot_{blk}_{u}")
                otp = otw[:].bitcast(BF16)  # [128, 1024] bf16 view
                for hb in range(2):
                    nc.tensor.transpose(
                        otp[:, 128 * hb:128 * hb + 98],
                        onorm[0:98, 128 * hb:128 * (hb + 1)],
                        idb_sb[0:98, 0:98])
                otsb = ot_pool.tile([128, 232], BF16, tag="otsb")
                otsb_v = otsb[:, 0:196].rearrange("p (b c) -> p b c", b=2)
                otp_v = otp[:, 0:256].rearrange("p (b c) -> p b c", b=2)[:, :, 0:98]
                nc.scalar.copy(otsb_v, otp_v)
                nc.vector.memset(otsb[:, 196:226], 0.0)
                # proj (lhsT slots packed 98 apart, 128-wide reads)
                pw4 = ps_w.tile([128, 512], F32, tag="work", name=f"pj_{blk}_{u}")
                for hb in range(2):
                    nc.tensor.matmul(pw4[:, 0:256], otsb[:, 98 * hb:98 * hb + 128],
                                     pwt_sb[:, 256 * hb:256 * (hb + 1)],
                                     start=(hb == 0), stop=(hb == 1))
                osb = out_pool.tile([128, 256], F32, tag="outsb")
                if pb_zero:
                    nc.scalar.copy(osb[0:98, :], pw4[0:98, 0:256])
                else:
                    nc.vector.tensor_add(osb[0:98, :], pw4[0:98, 0:256], pbb_sb[0:98, :])
                nc.sync.dma_start(out_ap[t0 + c0:t0 + c0 + 98, :], osb[0:98, :])


_CACHED = {}


def _get_program(mask_zero: bool, pb_zero: bool):
    key = (mask_zero, pb_zero, KSTAGE)
    if key in _CACHED:
        return _CACHED[key]
    nc = bacc.Bacc("TRN2", target_bir_lowering=False, debug=False)
    ins = {
        "xbf": nc.dram_tensor("xbf", [2, 128, T_CORE], BF16, kind="ExternalInput").ap(),
        "qkw": nc.dram_tensor("qkw", [128, 2, 4, 128], BF16, kind="ExternalInput").ap(),
        "wv": nc.dram_tensor("wv", [128, 2, 264], BF16, kind="ExternalInput").ap(),
        "pwt": nc.dram_tensor("pwt", [128, 2, 256], BF16, kind="ExternalInput").ap(),
        "ebias": nc.dram_tensor("ebias", [128, 8, 98], BF16, kind="ExternalInput").ap(),
        "identb": nc.dram_tensor("identb", [128, 128], BF16, kind="ExternalInput").ap(),
        "qkvb": nc.dram_tensor("qkvb", [128, 4], F32, kind="ExternalInput").ap(),
        "vbb": nc.dram_tensor("vbb", [128, 264], F32, kind="ExternalInput").ap(),
    }
    if not mask_zero:
        ins["expbm"] = nc.dram_tensor("expbm", [128, 32 * 784], BF16,
                                      kind="ExternalInput").ap()
    if not pb_zero:
        ins["pbb"] = nc.dram_tensor("pbb", [128, 256], F32, kind="ExternalInput").ap()
    out_ap = nc.dram_tensor("out", [T_CORE, DIM], F32, kind="ExternalOutput").ap()
    with tile.TileContext(nc) as tc:
        with ExitStack() as ctx:
            build_kernel(ctx, tc, ins, out_ap, mask_zero, pb_zero)
    nc.compile()
    _CACHED[key] = nc
    return nc


def _host_prep(mask, qkv_w, qkv_b, proj_w, proj_b, bias_table):
    bf = ml_dtypes.bfloat16
    qkv_w = np.asarray(qkv_w, np.float32)
    qkv_b = np.asarray(qkv_b, np.float32)
    mask = np.asarray(mask, np.float32)
    mask_zero = not np.any(mask)
    pb = np.asarray(proj_b, np.float32)
    pb_zero = not np.any(pb)

    wqk = qkv_w[0:512].copy()          # [512 oc, 256 ic]
    wqk[0:256] *= SCALE                # fold softmax scale into q
    # [ic, oc] -> [kb, 128ic, ob, 128oc] -> [128ic, kb, ob, 128oc]
    qkw = np.ascontiguousarray(
        wqk.T.reshape(2, 128, 4, 128).transpose(1, 0, 2, 3)).astype(bf)
    qb = qkv_b.copy()
    qb[0:256] *= SCALE
    qkvb = np.ascontiguousarray(qb[0:512].reshape(4, 128).T)   # [128, 4]

    wvT = qkv_w[512:768].T             # [256 ic, 256 oc]
    wv = np.zeros((2, 128, 264), np.float32)
    for h in range(NH):
        wv[:, :, 33 * h:33 * h + 32] = wvT.reshape(2, 128, 8, 32)[:, :, h]
    wv = np.ascontiguousarray(wv.transpose(1, 0, 2)).astype(bf)
    vb = np.zeros((264,), np.float32)
    for h in range(NH):
        vb[33 * h:33 * h + 32] = qkv_b[512 + 32 * h:512 + 32 * h + 32]
    vbb = np.ascontiguousarray(np.broadcast_to(vb, (128, 264)))

    pwt = np.ascontiguousarray(
        np.asarray(proj_w, np.float32).T.reshape(2, 128, 256)
        .transpose(1, 0, 2)).astype(bf)
    pbb = np.ascontiguousarray(np.broadcast_to(pb, (128, 256)))

    rel = _rel_pos_index()
    bias_g = np.asarray(bias_table, np.float32)[rel].reshape(NTOK, NTOK, NH)  # [i,j,h]
    ebT = np.exp(bias_g).transpose(1, 2, 0)          # [j, h, i]
    # device pm slot s = 2*(h%4) + h//4  ->  head order [0,4,1,5,2,6,3,7]
    SLOT_ORDER = [0, 4, 1, 5, 2, 6, 3, 7]
    ebias = np.zeros((128, 8, 98), np.float32)
    ebias[0:49, :, 0:49] = ebT
    ebias[49:98, :, 49:98] = ebT
    ebias = np.ascontiguousarray(ebias[:, SLOT_ORDER, :]).astype(bf)

    expbm = None
    if not mask_zero:
        expbm = np.zeros((128, 32, 8, 98), np.float32)
        for p in range(32):
            for w in range(2):
                cb = np.exp(bias_g + mask[2 * p + w][:, :, None]).transpose(1, 2, 0)
                expbm[49 * w:49 * w + 49, p, :, 49 * w:49 * w + 49] = cb
        expbm = np.ascontiguousarray(
            expbm[:, :, SLOT_ORDER, :].reshape(128, 32 * 784)).astype(bf)
    identb = np.eye(128).astype(bf)
    return (qkw, qkvb, wv, vbb, pwt, pbb, ebias, expbm, identb,
            mask_zero, pb_zero)


def kernel(x, mask, qkv_w, qkv_b, proj_w, proj_b, bias_table, _trace=False):
    bf = ml_dtypes.bfloat16
    (qkw, qkvb, wv, vbb, pwt, pbb, ebias, expbm, identb,
     mask_zero, pb_zero) = _host_prep(mask, qkv_w, qkv_b, proj_w, proj_b, bias_table)
    # [T_full, 256] -> per-core [2, 128, T_CORE] bf16 (x.T, kb-major)
    xT = np.asarray(x, np.float32).reshape(B_FULL * NTOK, DIM).T.astype(bf)
    xTs = xT.reshape(2, 128, NCORES, T_CORE)
    in_maps = []
    for c in range(NCORES):
        shard = np.ascontiguousarray(xTs[:, :, c, :])
        m = {"xbf": shard, "qkw": qkw, "wv": wv, "pwt": pwt, "ebias": ebias,
             "identb": identb, "qkvb": qkvb, "vbb": vbb}
        if not mask_zero:
            m["expbm"] = expbm
        if not pb_zero:
            m["pbb"] = pbb
        in_maps.append(m)
    nc = _get_program(mask_zero, pb_zero)
    res = bass_utils.run_bass_kernel_spmd(nc, in_maps, core_ids=list(range(NCORES)),
                                          trace=_trace)
    out = np.stack([r["out"] for r in res.results])  # [8, T_CORE, 256]
    out = out.reshape(B_FULL, NTOK, DIM)
    if _trace:
        kernel.last_results = res
    return out


# revision 35
# speedup vs baseline: 1.7728x; 1.2733x over previous
"""Trainium2 Bass kernel for Swin-style window attention (MegatronWindowAttention).

Per window w (49 tokens, dim 256, 8 heads x 32):
  qkv = x @ qkv_w.T + qkv_b ; attn = softmax(q*scale @ k.T + bias + mask[w%64])
  out = (attn @ v) @ proj_w.T + proj_b

Sharding: data-parallel over B_=4096 windows across 8 cores (512 windows each).

Device dataflow per core (4 blocks of 128 windows; window PAIRS of 98 tokens):
  - x.T bf16 arrives via DMA xbar transpose (host passes bf16 copy of x)
  - Q.T/K.T staged per 8-pair group: W-stationary matmuls (FWL 128-col weights)
  - V per pair in [token, channel] layout: x.T-chunk-stationary matmuls
    streaming Wv.T (slotted 33-wide per head, ones col for rowsum)
  - S.T per (head, pair): one matmul, lhsT = K.T 128-col span (both windows
    compact at partitions 0:98), rhs = Q.T 98 cols; exp on ACT; * exp(bias)
    table (zeros kill cross-window garbage quadrants) on DVE
  - AV per head: lhsT = pm 128-col slot (FWL), rhs = V_aug -> O + rowsum
  - normalize via DVE reciprocal+mul; 2 PE transposes -> O.T; proj 2 matmuls
"""

import os
import numpy as np
import ml_dtypes
from contextlib import ExitStack

KSTAGE = int(os.environ.get("KSTAGE", "5"))

import concourse.bass as bass
import concourse.tile as tile
import concourse.mybir as mybir
from concourse import bacc
from concourse import bass_utils

WH = WW = 7
NTOK = 49
DIM = 256
NH = 8
HD = 32
SCALE = HD ** -0.5
NCORES = 8
B_FULL = 4096
NW = 64
B_CORE = B_FULL // NCORES          # 512 windows per core
T_CORE = B_CORE * NTOK             # 25088 tokens
NBLOCK = 4
W_BLK = 128                        # windows per block
T_BLK = W_BLK * NTOK               # 6272 tokens per block
T_PAD = T_BLK + 32                 # tail pad so 128-wide lhsT reads stay in-bounds
NPAIR = W_BLK // 2                 # 64 pairs per block
GPAIR = 8                          # pairs per QK staging group
NGRP = NPAIR // GPAIR              # 8 groups per block
GW = GPAIR * 98                    # 784 tokens per group
GWH = GW + 32                      # +halo for K 128-col lhsT reads
GCH = GWH // 2                     # 408: QK psum chunk width

F32 = mybir.dt.float32
BF16 = mybir.dt.bfloat16
AF = mybir.ActivationFunctionType
ALU = mybir.AluOpType


def _rel_pos_index():
    coords = np.stack(np.meshgrid(np.arange(WH), np.arange(WW), indexing='ij'))
    flat = coords.reshape(2, -1)
    rel = flat[:, :, None] - flat[:, None, :]
    rel = rel.transpose(1, 2, 0).copy()
    rel[:, :, 0] += WH - 1
    rel[:, :, 1] += WW - 1
    rel[:, :, 0] *= 2 * WW - 1
    return rel.sum(-1).reshape(-1)


def build_kernel(ctx: ExitStack, tc: tile.TileContext, ins: dict, out_ap: bass.AP,
                 mask_zero: bool, pb_zero: bool):
    nc = tc.nc
    xbf = ins["xbf"]        # [T_CORE, 256] bf16
    qkw = ins["qkw"]        # [128, 2, 4, 128] bf16 (ic, kb, ob(q0 q1 k0 k1), oc)
    wv = ins["wv"]          # [128, 2, 264] bf16 (ic, kb, slotted oc)
    pwt = ins["pwt"]        # [128, 2, 256] bf16
    ebias = ins["ebias"]    # [128, 8, 98] bf16 (fast) or placeholder
    idb = ins["identb"]     # [128, 128] bf16
    qkvb = ins["qkvb"]      # [128, 4] f32 per-partition bias for q0 q1 k0 k1
    vbb = ins["vbb"]        # [128, 264] f32 broadcast v-bias (slotted)
    expbm = ins.get("expbm")  # [128, 32*784] bf16 (general path only)
    pbb = ins.get("pbb")    # [128, 256] f32 (general path only)

    const = ctx.enter_context(tc.tile_pool(name="const", bufs=1))
    qkw_sb = const.tile([128, 2 * 4 * 128], BF16, tag="qkw")
    nc.sync.dma_start(qkw_sb[:], qkw.rearrange("p a b c -> p (a b c)"))
    qkw_v = qkw_sb[:].rearrange("p (a b c) -> p a b c", a=2, b=4)
    wv_sb = const.tile([128, 2 * 264], BF16, tag="wv")
    nc.sync.dma_start(wv_sb[:], wv.rearrange("p a c -> p (a c)"))
    wv_v = wv_sb[:].rearrange("p (a c) -> p a c", a=2)
    pwt_sb = const.tile([128, 2 * 256], BF16, tag="pwt")
    nc.sync.dma_start(pwt_sb[:], pwt.rearrange("p a c -> p (a c)"))
    idb_sb = const.tile([128, 128], BF16, tag="identb")
    nc.sync.dma_start(idb_sb[:], idb[:])
    qkvb_sb = const.tile([128, 4], F32, tag="qkvb")
    nc.sync.dma_start(qkvb_sb[:], qkvb[:])
    vbb_sb = const.tile([128, 264], F32, tag="vbb")
    nc.sync.dma_start(vbb_sb[:], vbb[:])
    if mask_zero:
        eb_sb = const.tile([128, 8 * 98], BF16, tag="ebias")
        nc.sync.dma_start(eb_sb[:], ebias.rearrange("p a c -> p (a c)"))
    else:
        eb_sb = const.tile([128, 32 * 784], BF16, tag="expbm")
        nc.sync.dma_start(eb_sb[:], expbm[:])
    if not pb_zero:
        pbb_sb = const.tile([128, 256], F32, tag="pbb")
        nc.sync.dma_start(pbb_sb[:], pbb[:])

    xt_pool = ctx.enter_context(tc.tile_pool(name="xt", bufs=2))
    qk_pool = ctx.enter_context(tc.tile_pool(name="qk", bufs=2))
    pm_pool = ctx.enter_context(tc.tile_pool(name="pm", bufs=2))
    v_pool = ctx.enter_context(tc.tile_pool(name="vsb", bufs=2))
    r_pool = ctx.enter_context(tc.tile_pool(name="rsb", bufs=2))
    on_pool = ctx.enter_context(tc.tile_pool(name="onorm", bufs=2))
    ot_pool = ctx.enter_context(tc.tile_pool(name="otsb", bufs=2))
    out_pool = ctx.enter_context(tc.tile_pool(name="outsb", bufs=3))

    # 8 PSUM banks: qkp 2 (also proj), sps 4 (2 banks x 2 bufs, also O.T), work 2
    ps_qk = ctx.enter_context(tc.tile_pool(name="psqk", bufs=2, space="PSUM"))
    ps_s = ctx.enter_context(tc.tile_pool(name="pss", bufs=1, space="PSUM"))
    ps_w = ctx.enter_context(tc.tile_pool(name="psw", bufs=2, space="PSUM"))


    # back half of a pair: AV, normalize, O.T, proj, output. Emitted one pair
    # late so its PE work overlaps the next pair's exp on ACT.
    prev = None

    def emit_back(st):
        pm, vsb, dst = st["pm"], st["vsb"], st["dst"]
        avw = ps_w.tile([128, 512], F32, tag="work", name="av_" + st["tag"])
        for h in range(NH):
            s_h = 2 * (h % 4) + h // 4
            nc.tensor.matmul(
                avw[:, 33 * h:33 * h + 33],
                pm[0:98, 98 * s_h:98 * s_h + 128],
                vsb[0:98, 33 * h:33 * h + 33], start=True, stop=True)
        av_v = avw[:, 0:264].rearrange("p (h c) -> p h c", h=8)
        recip = r_pool.tile([128, 8], F32, tag="recip")
        nc.vector.reciprocal(recip[0:98, :], av_v[0:98, :, 32])
        onorm = on_pool.tile([128, 256], BF16, tag="onorm")
        onorm_v = onorm[:].rearrange("p (h c) -> p h c", h=8)
        recip_b = recip[0:98, :].unsqueeze(2).broadcast_to([98, 8, 32])
        nc.vector.tensor_mul(onorm_v[0:98, :, :], av_v[0:98, :, 0:32], recip_b)
        # O.T via 2 PE transposes -> work psum -> sbuf (98-packed slots)
        otw = ps_w.tile([128, 512], F32, tag="work", name="ot_" + st["tag"])
        otp = otw[:].bitcast(BF16)
        for hb in range(2):
            nc.tensor.transpose(
                otp[:, 98 * hb:98 * hb + 98],
                onorm[0:98, 128 * hb:128 * (hb + 1)],
                idb_sb[0:98, 0:98])
        otsb = ot_pool.tile([128, 232], BF16, tag="otsb")
        otsb_v = otsb[:, 0:196].rearrange("p (b c) -> p b c", b=2)
        otp_v = otp[:, 0:196].rearrange("p (b c) -> p b c", b=2)
        nc.scalar.copy(otsb_v, otp_v)
        nc.vector.memset(otsb[:, 196:226], 0.0)
        # proj (lhsT slots packed 98 apart, 128-wide reads); psum from the
        # qkp pool, which is idle during the pair stream
        pw4 = ps_qk.tile([128, 512], F32, tag="qkp", name="pj_" + st["tag"])
        for hb in range(2):
            nc.tensor.matmul(pw4[:, 0:256], otsb[:, 98 * hb:98 * hb + 128],
                             pwt_sb[:, 256 * hb:256 * (hb + 1)],
                             start=(hb == 0), stop=(hb == 1))
        osb = out_pool.tile([128, 256], F32, tag="outsb")
        if pb_zero:
            nc.scalar.copy(osb[0:98, :], pw4[0:98, 0:256])
        else:
            nc.vector.tensor_add(osb[0:98, :], pw4[0:98, 0:256], pbb_sb[0:98, :])
        nc.sync.dma_start(dst, osb[0:98, :])

    for blk in range(NBLOCK):
        t0 = blk * T_BLK
        # ---- x.T loaded directly (host pre-transposed); zero the tail pad ----
        Xt = [xt_pool.tile([128, T_PAD], BF16, tag=f"xt{kb}", name=f"xt{kb}_{blk}")
              for kb in range(2)]
        for kb in range(2):
            nc.vector.memset(Xt[kb][:, T_BLK:T_PAD], 0.0)
            nc.sync.dma_start(Xt[kb][:, 0:T_BLK], xbf[kb, :, t0:t0 + T_BLK])
        for grp in range(NGRP):
            g0 = GW * grp
            # ---- Q.T/K.T staging for this group (halo of 32 for K reads) ----
            # last group's halo reads the zeroed pad region
            qks = [qk_pool.tile([128, GWH], BF16, tag=f"qk{ob}", name=f"qk{ob}_{blk}_{grp}")
                   for ob in range(4)]
            for c2 in range(2):
                cs = slice(g0 + GCH * c2, g0 + GCH * (c2 + 1))
                for ob in range(4):
                    qkp = ps_qk.tile([128, 512], F32, tag="qkp")
                    for kb in range(2):
                        nc.tensor.matmul(qkp[:, 0:GCH], qkw_v[:, kb, ob, :],
                                         Xt[kb][:, cs],
                                         start=(kb == 0), stop=(kb == 1))
                    nc.vector.tensor_scalar_add(
                        qks[ob][:, GCH * c2:GCH * (c2 + 1)], qkp[:, 0:GCH],
                        qkvb_sb[:, ob:ob + 1])
            # ---- attention pairs (software-skewed pipeline) ----
            for p8 in range(GPAIR):
                u = GPAIR * grp + p8
                c0 = 98 * u            # block-local token offset
                cg = 98 * p8           # group-local token offset
                # ---- front half of pair u: S, V, exp, pm ----
                # S.T: one matmul per head, both windows (keys compact 0:98).
                # Concurrent row-bands must use distinct PSUM banks: band
                # h%4 -> bank h%4; heads h, h+4 share a band (serialized by
                # the PE) and pack side by side within the bank.
                sps = ps_s.tile([128, 2048], F32, tag="sps")
                for h in range(NH):
                    kt = qks[2 + h // 4]
                    qt = qks[0 + h // 4]
                    rs = slice(32 * (h % 4), 32 * (h % 4) + 32)
                    sc = 512 * (h % 4) + 98 * (h // 4)
                    nc.tensor.matmul(
                        sps[:, sc:sc + 98],
                        kt[rs, cg:cg + 128], qt[rs, cg:cg + 98],
                        start=True, stop=True, tile_position=(32 * (h % 4), 0))
                # V in [token, channel-slot] layout: x.T chunk stationary
                vw = ps_w.tile([128, 512], F32, tag="work", name=f"v_{blk}_{u}")
                for kb in range(2):
                    nc.tensor.matmul(vw[:, 0:264], Xt[kb][:, c0:c0 + 128],
                                     wv_v[:, kb, :], start=(kb == 0), stop=(kb == 1))
                vsb = v_pool.tile([128, 264], BF16, tag="vsb")
                nc.vector.tensor_add(vsb[0:98, :], vw[0:98, 0:264], vbb_sb[0:98, :])
                vsb_v = vsb[:].rearrange("p (h c) -> p h c", h=8)
                nc.vector.memset(vsb_v[0:98, :, 32:33], 1.0)
                # exp (ACT) then * exp(bias[+mask]) (DVE, zeros kill garbage).
                # pm slot s = 2*(h%4) + h//4, packed 98 apart: AV's 128-wide
                # lhsT reads spill into the next slot's valid data (garbage ->
                # unused out partitions 98:128); last slot gets a tail memset.
                pex = pm_pool.tile([128, 784], BF16, tag="pex")
                pex_v = pex[:].rearrange("p (r c) -> p r c", r=4)
                sps_v = sps[:].rearrange("p (r c) -> p r c", r=4)[:, :, 0:196]
                nc.scalar.activation(pex_v, sps_v, AF.Exp)
                pm = pm_pool.tile([128, 832], BF16, tag="pm")
                pm_v = pm[:, 0:784].rearrange("p (r c) -> p r c", r=4)
                nc.vector.memset(pm[:, 784:814], 0.0)
                if mask_zero:
                    ebv = eb_sb[:].rearrange("p (r c) -> p r c", r=4)
                else:
                    ebv = eb_sb[:, 784 * (u % 32):784 * (u % 32 + 1)].rearrange(
                        "p (r c) -> p r c", r=4)
                nc.vector.tensor_mul(pm_v, pex_v, ebv)
                # ---- back half of the PREVIOUS pair ----
                if prev is not None:
                    emit_back(prev)
                prev = dict(pm=pm, vsb=vsb, dst=out_ap[t0 + c0:t0 + c0 + 98, :],
                            tag=f"{blk}_{u}")
    if prev is not None:
        emit_back(prev)


def _noop():
    pass


_CACHED = {}


def _get_program(mask_zero: bool, pb_zero: bool):
    key = (mask_zero, pb_zero)
    if key in _CACHED:
        return _CACHED[key]
    nc = bacc.Bacc("TRN2", target_bir_lowering=False, debug=False)
    ins = {
        "xbf": nc.dram_tensor("xbf", [2, 128, T_CORE], BF16, kind="ExternalInput").ap(),
        "qkw": nc.dram_tensor("qkw", [128, 2, 4, 128], BF16, kind="ExternalInput").ap(),
        "wv": nc.dram_tensor("wv", [128, 2, 264], BF16, kind="ExternalInput").ap(),
        "pwt": nc.dram_tensor("pwt", [128, 2, 256], BF16, kind="ExternalInput").ap(),
        "ebias": nc.dram_tensor("ebias", [128, 8, 98], BF16, kind="ExternalInput").ap(),
        "identb": nc.dram_tensor("identb", [128, 128], BF16, kind="ExternalInput").ap(),
        "qkvb": nc.dram_tensor("qkvb", [128, 4], F32, kind="ExternalInput").ap(),
        "vbb": nc.dram_tensor("vbb", [128, 264], F32, kind="ExternalInput").ap(),
    }
    if not mask_zero:
        ins["expbm"] = nc.dram_tensor("expbm", [128, 32 * 784], BF16,
                                      kind="ExternalInput").ap()
    if not pb_zero:
        ins["pbb"] = nc.dram_tensor("pbb", [128, 256], F32, kind="ExternalInput").ap()
    out_ap = nc.dram_tensor("out", [T_CORE, DIM], F32, kind="ExternalOutput").ap()
    with tile.TileContext(nc) as tc:
        with ExitStack() as ctx:
            build_kernel(ctx, tc, ins, out_ap, mask_zero, pb_zero)
    nc.compile()
    _CACHED[key] = nc
    return nc


def _host_prep(mask, qkv_w, qkv_b, proj_w, proj_b, bias_table):
    bf = ml_dtypes.bfloat16
    qkv_w = np.asarray(qkv_w, np.float32)
    qkv_b = np.asarray(qkv_b, np.float32)
    mask = np.asarray(mask, np.float32)
    mask_zero = not np.any(mask)
    pb = np.asarray(proj_b, np.float32)
    pb_zero = not np.any(pb)

    wqk = qkv_w[0:512].copy()          # [512 oc, 256 ic]
    wqk[0:256] *= SCALE                # fold softmax scale into q
    # [ic, oc] -> [kb, 128ic, ob, 128oc] -> [128ic, kb, ob, 128oc]
    qkw = np.ascontiguousarray(
        wqk.T.reshape(2, 128, 4, 128).transpose(1, 0, 2, 3)).astype(bf)
    qb = qkv_b.copy()
    qb[0:256] *= SCALE
    qkvb = np.ascontiguousarray(qb[0:512].reshape(4, 128).T)   # [128, 4]

    wvT = qkv_w[512:768].T             # [256 ic, 256 oc]
    wv = np.zeros((2, 128, 264), np.float32)
    for h in range(NH):
        wv[:, :, 33 * h:33 * h + 32] = wvT.reshape(2, 128, 8, 32)[:, :, h]
    wv = np.ascontiguousarray(wv.transpose(1, 0, 2)).astype(bf)
    vb = np.zeros((264,), np.float32)
    for h in range(NH):
        vb[33 * h:33 * h + 32] = qkv_b[512 + 32 * h:512 + 32 * h + 32]
    vbb = np.ascontiguousarray(np.broadcast_to(vb, (128, 264)))

    pwt = np.ascontiguousarray(
        np.asarray(proj_w, np.float32).T.reshape(2, 128, 256)
        .transpose(1, 0, 2)).astype(bf)
    pbb = np.ascontiguousarray(np.broadcast_to(pb, (128, 256)))

    rel = _rel_pos_index()
    bias_g = np.asarray(bias_table, np.float32)[rel].reshape(NTOK, NTOK, NH)  # [i,j,h]
    ebT = np.exp(bias_g).transpose(1, 2, 0)          # [j, h, i]
    # device pm slot s = 2*(h%4) + h//4 -> head order [0,4,1,5,2,6,3,7]
    SLOT_ORDER = [0, 4, 1, 5, 2, 6, 3, 7]
    ebias = np.zeros((128, 8, 98), np.float32)
    ebias[0:49, :, 0:49] = ebT
    ebias[49:98, :, 49:98] = ebT
    ebias = np.ascontiguousarray(ebias[:, SLOT_ORDER, :]).astype(bf)

    expbm = None
    if not mask_zero:
        expbm = np.zeros((128, 32, 8, 98), np.float32)
        for p in range(32):
            for w in range(2):
                cb = np.exp(bias_g + mask[2 * p + w][:, :, None]).transpose(1, 2, 0)
                expbm[49 * w:49 * w + 49, p, :, 49 * w:49 * w + 49] = cb
        expbm = np.ascontiguousarray(
            expbm[:, :, SLOT_ORDER, :].reshape(128, 32 * 784)).astype(bf)
    identb = np.eye(128).astype(bf)
    return (qkw, qkvb, wv, vbb, pwt, pbb, ebias, expbm, identb,
            mask_zero, pb_zero)


def kernel(x, mask, qkv_w, qkv_b, proj_w, proj_b, bias_table, _trace=False):
    bf = ml_dtypes.bfloat16
    (qkw, qkvb, wv, vbb, pwt, pbb, ebias, expbm, identb,
     mask_zero, pb_zero) = _host_prep(mask, qkv_w, qkv_b, proj_w, proj_b, bias_table)
    # [T_full, 256] -> per-core [2, 128, T_CORE] bf16 (x.T, kb-major)
    xT = np.asarray(x, np.float32).reshape(B_FULL * NTOK, DIM).T.astype(bf)
    xTs = xT.reshape(2, 128, NCORES, T_CORE)
    in_maps = []
    for c in range(NCORES):
        shard = np.ascontiguousarray(xTs[:, :, c, :])
        m = {"xbf": shard, "qkw": qkw, "wv": wv, "pwt": pwt, "ebias": ebias,
             "identb": identb, "qkvb": qkvb, "vbb": vbb}
        if not mask_zero:
            m["expbm"] = expbm
        if not pb_zero:
            m["pbb"] = pbb
        in_maps.append(m)
    nc = _get_program(mask_zero, pb_zero)
    res = bass_utils.run_bass_kernel_spmd(nc, in_maps, core_ids=list(range(NCORES)),
                                          trace=_trace)
    out = np.stack([r["out"] for r in res.results])  # [8, T_CORE, 256]
    out = out.reshape(B_FULL, NTOK, DIM)
    if _trace:
        kernel.last_results = res
    return out
